# revision 1
# baseline (speedup 1.0000x reference)
"""GRU-D decoder kernel for Trainium2 (8 NeuronCores, data-parallel over batch).

Math (mask == ones everywhere, which the reference hardcodes):
  x_hat = C (constant), d = dt broadcast, gamma_x unused.
  gamma[t,b,j] = exp(-relu(dt[t,b] * colsum(Wgh)[j] + bgh[j]))   (precomputed host-side)
  per step: hdec = gamma_t * h
            z = sigmoid(hdec @ Wz_h + Az0);  r = sigmoid(hdec @ Wr_h + Ar0)
            htl = tanh((r*hdec) @ Wh_h + Ah0)
            h = hdec + z*(htl - hdec)
  out[t] = h_t @ Wlin + blin
  where A?0 = C @ W?_x + colsum(W?_m) + b?  (time-constant, precomputed host-side).

Device layout: everything transposed (H on partitions as 4 tiles of 128,
batch=64 on the free dim), packed as SBUF tiles (128, 4*64) with column
index = kt*64 + b.  Gate matmuls use the weight blocks as stationary
operands and hdec slices as moving operands; outputs land natively in the
same transposed layout, so no transposes are needed anywhere.  The
per-step tail (tanh/blend/decay) is split into two column halves so the
tensor engine can start the next group while the tail of the previous
half is still on Scalar/Vector.
"""

import numpy as np
import ml_dtypes

T, B, H, O = 100, 512, 512, 512
NCORES = 8
BL = B // NCORES  # 64
KC = 4  # contraction chunks of 128
JT = 4  # output j-tiles of 128
FR = JT * BL  # 256
HB = FR // 2  # 128 (half of the free dim; = 2 j-tiles)
GCH = 20  # gamma chunk (steps per DMA)

_BUILD_CACHE = {}


def _build_program():
    if "nc" in _BUILD_CACHE:
        return _BUILD_CACHE["nc"]

    import concourse.tile as tile
    import concourse.mybir as mybir
    from concourse import bacc
    from contextlib import ExitStack

    f32 = mybir.dt.float32
    bf16 = mybir.dt.bfloat16
    AF = mybir.ActivationFunctionType

    nc = bacc.Bacc("TRN2", target_bir_lowering=False, debug=False,
                   num_devices=NCORES)

    gam_d = nc.dram_tensor("gam", [128, T, FR], f32, kind="ExternalInput")
    wzr_d = nc.dram_tensor("wzr", [128, KC * 2 * JT * 128], bf16, kind="ExternalInput")
    wht_d = nc.dram_tensor("wht", [128, KC * JT * 128], bf16, kind="ExternalInput")
    wlin_d = nc.dram_tensor("wlin", [128, KC * O], bf16, kind="ExternalInput")
    a0z_d = nc.dram_tensor("a0z", [128, FR], bf16, kind="ExternalInput")
    a0r_d = nc.dram_tensor("a0r", [128, FR], bf16, kind="ExternalInput")
    a0h_d = nc.dram_tensor("a0h", [128, FR], bf16, kind="ExternalInput")
    ident_d = nc.dram_tensor("ident", [128, 128], bf16, kind="ExternalInput")
    ones_d = nc.dram_tensor("ones64", [1, BL], bf16, kind="ExternalInput")
    blinr_d = nc.dram_tensor("blinr", [1, O], bf16, kind="ExternalInput")
    out_d = nc.dram_tensor("out", [T, BL, O], f32, kind="ExternalOutput")

    with tile.TileContext(nc) as tc, ExitStack() as ctx:
        constp = ctx.enter_context(tc.tile_pool(name="const", bufs=1))
        gpool = ctx.enter_context(tc.tile_pool(name="gam", bufs=2))
        statep = ctx.enter_context(tc.tile_pool(name="state", bufs=1))
        hdp = ctx.enter_context(tc.tile_pool(name="hd", bufs=2))
        actp = ctx.enter_context(tc.tile_pool(name="act", bufs=2))
        pzp = ctx.enter_context(tc.tile_pool(name="pz", bufs=1, space="PSUM"))
        prp = ctx.enter_context(tc.tile_pool(name="pr", bufs=1, space="PSUM"))
        php0 = ctx.enter_context(tc.tile_pool(name="ph0", bufs=1, space="PSUM"))
        php1 = ctx.enter_context(tc.tile_pool(name="ph1", bufs=1, space="PSUM"))
        pjp = ctx.enter_context(tc.tile_pool(name="pj", bufs=2, space="PSUM"))

        wzr = constp.tile([128, KC * 2 * JT * 128], bf16)
        nc.sync.dma_start(wzr[:], wzr_d[:])
        wht = constp.tile([128, KC * JT * 128], bf16)
        nc.sync.dma_start(wht[:], wht_d[:])
        wlin = constp.tile([128, KC * O], bf16)
        nc.sync.dma_start(wlin[:], wlin_d[:])
        a0z = constp.tile([128, FR], bf16)
        nc.sync.dma_start(a0z[:], a0z_d[:])
        a0r = constp.tile([128, FR], bf16)
        nc.sync.dma_start(a0r[:], a0r_d[:])
        a0h = constp.tile([128, FR], bf16)
        nc.sync.dma_start(a0h[:], a0h_d[:])
        ident = constp.tile([128, 128], bf16)
        nc.sync.dma_start(ident[:], ident_d[:])
        ones64 = constp.tile([1, BL], bf16)
        nc.sync.dma_start(ones64[:], ones_d[:])
        blinr = constp.tile([1, O], bf16)
        nc.sync.dma_start(blinr[:], blinr_d[:])

        h = statep.tile([128, FR], f32)
        nc.vector.memset(h[:], 0.0)

        def wzr_blk(g, jo, kc):
            i = ((kc * 2 + g) * JT + jo) * 128
            return wzr[:, i:i + 128]

        def wht_blk(jo, kc):
            i = (kc * JT + jo) * 128
            return wht[:, i:i + 128]

        # gamma chunks, preloaded half a chunk ahead
        chunks = {}

        def ensure_chunk(c):
            if c in chunks or c * GCH >= T:
                return
            t0 = c * GCH
            t1 = min(t0 + GCH, T)
            gt = gpool.tile([128, GCH * FR], f32, tag="gchunk")
            nc.sync.dma_start(gt[:, 0:(t1 - t0) * FR], gam_d[:, t0:t1, :])
            chunks[c] = gt

        def gamma_half(tt, hf):
            c2, o2 = divmod(tt, GCH)
            return chunks[c2][:, o2 * FR + hf * HB: o2 * FR + (hf + 1) * HB]

        ensure_chunk(0)

        # step-0 decayed state is zero
        hdf = hdp.tile([128, FR], f32, tag="hdf")
        nc.vector.memset(hdf[:], 0.0)
        hdb = hdp.tile([128, FR], bf16, tag="hdb")
        nc.vector.memset(hdb[:], 0.0)

        hbf_prev = None
        pj_prev = None

        for t in range(T):
            c, o = divmod(t, GCH)
            if o == GCH // 2:
                ensure_chunk(c + 1)

            # ---- output DMA for step t-1 (projection ran at the end of t-1)
            if pj_prev is not None:
                osb = actp.tile([BL, O], f32, tag="osb")
                nc.scalar.copy(osb[:], pj_prev[:])
                nc.sync.dma_start(out_d[t - 1], osb[:])

            # ---- r gate matmuls, jo-major: each pr j-slice completes after 4
            # MMs so sigmoid(r) halves start while later slices still run
            pr = prp.tile([128, FR], f32, tag="pr")
            nc.tensor.matmul(pr[:], ident[:], a0r[:], start=True, stop=False)
            for jo in range(JT):
                for kc in range(KC):
                    nc.tensor.matmul(
                        pr[:, jo * BL:(jo + 1) * BL],
                        wzr_blk(1, jo, kc),
                        hdb[:, kc * BL:(kc + 1) * BL],
                        start=False, stop=(kc == KC - 1),
                    )
            rb = actp.tile([128, FR], bf16, tag="rb")
            nc.scalar.activation(rb[:, 0:HB], pr[:, 0:HB], AF.Sigmoid)
            nc.scalar.activation(rb[:, HB:FR], pr[:, HB:FR], AF.Sigmoid)
            rh = hdp.tile([128, FR], bf16, tag="rh")
            nc.vector.tensor_mul(rh[:, 0:HB], rb[:, 0:HB], hdb[:, 0:HB])
            nc.vector.tensor_mul(rh[:, HB:FR], rb[:, HB:FR], hdb[:, HB:FR])

            # ---- z gate first half (jo 0,1)
            pz = pzp.tile([128, FR], f32, tag="pz")
            nc.tensor.matmul(pz[:], ident[:], a0z[:], start=True, stop=False)
            for jo in (0, 1):
                for kc in range(KC):
                    nc.tensor.matmul(
                        pz[:, jo * BL:(jo + 1) * BL],
                        wzr_blk(0, jo, kc),
                        hdb[:, kc * BL:(kc + 1) * BL],
                        start=False, stop=(kc == KC - 1),
                    )

            # ---- candidate gate, kc-chunks 0,1 (gated only by rh half 0)
            ph0 = php0.tile([128, HB], f32, tag="ph0")
            ph1 = php1.tile([128, HB], f32, tag="ph1")
            nc.tensor.matmul(ph0[:], ident[:], a0h[:, 0:HB], start=True, stop=False)
            nc.tensor.matmul(ph1[:], ident[:], a0h[:, HB:FR], start=True, stop=False)
            for kc in (0, 1):
                for jo in range(JT):
                    tgt = ph0 if jo < 2 else ph1
                    nc.tensor.matmul(
                        tgt[:, (jo % 2) * BL:(jo % 2 + 1) * BL],
                        wht_blk(jo, kc),
                        rh[:, kc * BL:(kc + 1) * BL],
                        start=False, stop=False,
                    )

            # ---- z gate second half (jo 2,3)
            for jo in (2, 3):
                for kc in range(KC):
                    nc.tensor.matmul(
                        pz[:, jo * BL:(jo + 1) * BL],
                        wzr_blk(0, jo, kc),
                        hdb[:, kc * BL:(kc + 1) * BL],
                        start=False, stop=(kc == KC - 1),
                    )
            zf = actp.tile([128, FR], f32, tag="zf")
            nc.scalar.activation(zf[:, 0:HB], pz[:, 0:HB], AF.Sigmoid)
            nc.scalar.activation(zf[:, HB:FR], pz[:, HB:FR], AF.Sigmoid)

            # ---- candidate gate, kc-chunks 2,3; jo 0,1 slices finish first so
            # tanh(half 0) can start while jo 2,3 still accumulate
            for jo in (0, 1):
                nc.tensor.matmul(
                    ph0[:, jo * BL:(jo + 1) * BL], wht_blk(jo, 2),
                    rh[:, 2 * BL:3 * BL], start=False, stop=False)
                nc.tensor.matmul(
                    ph0[:, jo * BL:(jo + 1) * BL], wht_blk(jo, 3),
                    rh[:, 3 * BL:4 * BL], start=False, stop=True)
            for jo in (2, 3):
                nc.tensor.matmul(
                    ph1[:, (jo - 2) * BL:(jo - 1) * BL], wht_blk(jo, 2),
                    rh[:, 2 * BL:3 * BL], start=False, stop=False)
                nc.tensor.matmul(
                    ph1[:, (jo - 2) * BL:(jo - 1) * BL], wht_blk(jo, 3),
                    rh[:, 3 * BL:4 * BL], start=False, stop=True)

            # ---- blend: h = (1-z)*hdec + z*htl, with (1-z)*hdec computed
            # off the tanh critical path
            zm = actp.tile([128, FR], f32, tag="zm")
            nc.vector.tensor_scalar(zm[:, 0:HB], zf[:, 0:HB], -1.0, 1.0,
                                    mybir.AluOpType.mult, mybir.AluOpType.add)
            pp_ = actp.tile([128, FR], f32, tag="pp")
            nc.vector.tensor_mul(pp_[:, 0:HB], zm[:, 0:HB], hdf[:, 0:HB])
            nc.vector.tensor_scalar(zm[:, HB:FR], zf[:, HB:FR], -1.0, 1.0,
                                    mybir.AluOpType.mult, mybir.AluOpType.add)
            nc.vector.tensor_mul(pp_[:, HB:FR], zm[:, HB:FR], hdf[:, HB:FR])

            hdf_n = hdb_n = None
            if t + 1 < T:
                hdf_n = hdp.tile([128, FR], f32, tag="hdf")
                hdb_n = hdp.tile([128, FR], bf16, tag="hdb")
            for hf, ph in ((0, ph0), (1, ph1)):
                sl = slice(hf * HB, (hf + 1) * HB)
                htl = actp.tile([128, HB], f32, tag=f"htl{hf}")
                nc.scalar.activation(htl[:], ph[:], AF.Tanh)
                qq = actp.tile([128, HB], f32, tag=f"qq{hf}")
                nc.vector.tensor_mul(qq[:], zf[:, sl], htl[:])
                nc.vector.tensor_add(h[:, sl], qq[:], pp_[:, sl])
                if t + 1 < T:
                    # bf16 decayed state straight from the fp32 mul (cast on write)
                    nc.vector.tensor_mul(hdb_n[:, sl], gamma_half(t + 1, hf), h[:, sl])
            if t + 1 < T:
                nc.vector.tensor_mul(hdf_n[:], chunks[(t + 1) // GCH][
                    :, ((t + 1) % GCH) * FR:((t + 1) % GCH + 1) * FR], h[:])
                hdf, hdb = hdf_n, hdb_n

            hbf = actp.tile([128, FR], bf16, tag="hbf")
            nc.scalar.copy(hbf[:], h[:])
            hbf_prev = hbf

            # ---- projection of h(t) at the end of the PE stream (fills the
            # tanh/blend tail); DMA'd out at the start of step t+1
            pj_prev = pjp.tile([BL, O], f32, tag="pj")
            nc.tensor.matmul(pj_prev[:], ones64[:], blinr[:], start=True, stop=False)
            for kc in range(KC):
                nc.tensor.matmul(
                    pj_prev[:],
                    hbf_prev[:, kc * BL:(kc + 1) * BL],
                    wlin[:, kc * O:(kc + 1) * O],
                    start=False, stop=(kc == KC - 1),
                )

        osb = actp.tile([BL, O], f32, tag="osb")
        nc.scalar.copy(osb[:], pj_prev[:])
        nc.sync.dma_start(out_d[T - 1], osb[:])

    nc.compile()
    _BUILD_CACHE["nc"] = nc
    return nc


def _host_prep(C, t, Wz, bz, Wr, br, Wh, bh, Wgh, bgh, Wlin, blin):
    """Build per-core input maps (all the precomputed, packed device tensors)."""
    bf = ml_dtypes.bfloat16

    s = Wgh.sum(axis=0)  # (H,)
    t3 = t[:, :, 0]  # (T,B)
    dt = np.concatenate([np.zeros((1, B), np.float32), t3[1:] - t3[:-1]], axis=0)
    # gamma (T,B,H) fp32
    gam = np.exp(-np.maximum(dt[:, :, None] * s[None, None, :] + bgh[None, None, :], 0.0)).astype(np.float32)

    def gate_const(W, b):
        # C @ W_x + colsum(W_m) + b  -> (B,H)
        return C @ W[0:H] + (W[2 * H:3 * H].sum(axis=0) + b)[None, :]

    Az0 = gate_const(Wz, bz).astype(np.float32)
    Ar0 = gate_const(Wr, br).astype(np.float32)
    Ah0 = gate_const(Wh, bh).astype(np.float32)

    Wg = np.stack([Wz[H:2 * H], Wr[H:2 * H]])  # (2,H,H)
    # wzr packed: [k, (kc,g,jo,m)]
    wzr = Wg.reshape(2, KC, 128, JT, 128).transpose(2, 1, 0, 3, 4).reshape(128, KC * 2 * JT * 128)
    wht = Wh[H:2 * H].reshape(KC, 128, JT, 128).transpose(1, 0, 2, 3).reshape(128, KC * JT * 128)
    wlin = Wlin.reshape(KC, 128, O).transpose(1, 0, 2).reshape(128, KC * O)
    wzr = np.ascontiguousarray(wzr, dtype=bf)
    wht = np.ascontiguousarray(wht, dtype=bf)
    wlin = np.ascontiguousarray(wlin, dtype=bf)
    ident = np.eye(128, dtype=bf)

    in_maps = []
    for i in range(NCORES):
        sl = slice(i * BL, (i + 1) * BL)
        gf = gam[:, sl, :]  # (T,BL,H)
        # gam packed: [p, t, kt*BL+b]
        gp = np.ascontiguousarray(gf.reshape(T, BL, KC, 128).transpose(3, 0, 2, 1).reshape(128, T, KC * BL))

        def packA(A):
            return np.ascontiguousarray(
                A[sl].reshape(BL, JT, 128).transpose(2, 1, 0).reshape(128, JT * BL), dtype=bf)

        in_maps.append({
            "gam": gp,
            "wzr": wzr,
            "wht": wht,
            "wlin": wlin,
            "a0z": packA(Az0),
            "a0r": packA(Ar0),
            "a0h": packA(Ah0),
            "ident": ident,
            "ones64": np.ones((1, BL), dtype=bf),
            "blinr": np.ascontiguousarray(blin.reshape(1, O), dtype=bf),
        })
    return in_maps


def kernel(C, t, mask, Wz, bz, Wr, br, Wh, bh, Wgh, bgh, wgx, bgx, Wlin, blin,
           _trace=False, _trace_kwargs=None):
    C = np.asarray(C, np.float32)
    t = np.asarray(t, np.float32)
    nc = _build_program()
    in_maps = _host_prep(C, t,
                         np.asarray(Wz, np.float32), np.asarray(bz, np.float32),
                         np.asarray(Wr, np.float32), np.asarray(br, np.float32),
                         np.asarray(Wh, np.float32), np.asarray(bh, np.float32),
                         np.asarray(Wgh, np.float32), np.asarray(bgh, np.float32),
                         np.asarray(Wlin, np.float32), np.asarray(blin, np.float32))

    from concourse.bass_utils import run_bass_kernel_spmd
    res = run_bass_kernel_spmd(nc, in_maps, list(range(NCORES)),
                               trace=_trace, **(_trace_kwargs or {}))
    outs = [res.results[i]["out"] for i in range(NCORES)]
    full = np.concatenate(outs, axis=1).astype(np.float32)  # (T,B,O)
    kernel._last_results = res
    return full



# revision 4
# speedup vs baseline: 1.3777x; 1.3777x over previous
"""GRU-D decoder kernel for Trainium2 (8 NeuronCores, data-parallel over batch).

Math (mask == ones everywhere, which the reference hardcodes):
  x_hat = C (constant), d = dt broadcast, gamma_x unused.
  gamma[t,b,j] = exp(-relu(dt[t,b] * colsum(Wgh)[j] + bgh[j]))   (precomputed host-side)
  per step: hdec = gamma_t * h
            z = sigmoid(hdec @ Wz_h + Az0);  r = sigmoid(hdec @ Wr_h + Ar0)
            htl = tanh((r*hdec) @ Wh_h + Ah0)
            h = (1-z)*hdec + z*htl
  out[t] = h_t @ Wlin + blin
  where A?0 = C @ W?_x + colsum(W?_m) + b?  (time-constant, precomputed host-side).

Device layout: everything transposed (H on partitions as 4 tiles of 128,
batch=64 on the free dim), packed as (128, 4*64) tiles with column index
kt*64 + b.  Gate matmuls run kc-major (contraction-chunk outer) so the
next step's accumulation can begin as soon as the first j-chunks of the
new decayed state land.  The recurrent tail is all-bf16 on DVE with the
gamma products (a = g*(1-z)*hdec, c = g*z) computed off the tanh critical
path, so only two DVE ops separate tanh from the next step's matmuls.
The output projection is transposed (Wlin blocks stationary, out = o on
partitions) and batched over groups of 4 steps (free dim 256, full PE
efficiency); its matmuls are spread one contraction-round per step to
fill the PE stall while the tail of the previous step drains.  Outputs
are DMA'd in the packed layout and untransposed on the host.
"""

import numpy as np
import ml_dtypes

T, B, H, O = 100, 512, 512, 512
NCORES = 8
BL = B // NCORES  # 64
KC = 4  # contraction chunks of 128
JT = 4  # output j-tiles of 128
FR = JT * BL  # 256
HB = FR // 2  # 128 (half of the free dim; = 2 j-tiles)
GCH = 20  # gamma chunk (steps per DMA)
PG = 4  # projection group size (steps per batched projection)

_BUILD_CACHE = {}


def _build_program():
    if "nc" in _BUILD_CACHE:
        return _BUILD_CACHE["nc"]

    import concourse.tile as tile
    import concourse.mybir as mybir
    from concourse import bacc
    from contextlib import ExitStack

    f32 = mybir.dt.float32
    bf16 = mybir.dt.bfloat16
    AF = mybir.ActivationFunctionType

    nc = bacc.Bacc("TRN2", target_bir_lowering=False, debug=False,
                   num_devices=NCORES)

    gam_d = nc.dram_tensor("gam", [128, T, FR], bf16, kind="ExternalInput")
    wzr_d = nc.dram_tensor("wzr", [128, KC * 2 * JT * 128], bf16, kind="ExternalInput")
    wht_d = nc.dram_tensor("wht", [128, KC * JT * 128], bf16, kind="ExternalInput")
    wlin_d = nc.dram_tensor("wlin", [128, KC * JT * 128], bf16, kind="ExternalInput")
    a0zr_d = nc.dram_tensor("a0zr", [128, 2 * FR], bf16, kind="ExternalInput")
    a0h_d = nc.dram_tensor("a0h", [128, FR], bf16, kind="ExternalInput")
    blin01_d = nc.dram_tensor("blin01", [128, 2 * FR], bf16, kind="ExternalInput")
    blin23_d = nc.dram_tensor("blin23", [128, 2 * FR], bf16, kind="ExternalInput")
    ident_d = nc.dram_tensor("ident", [128, 128], bf16, kind="ExternalInput")
    out_d = nc.dram_tensor("out", [128, T * FR], f32, kind="ExternalOutput")

    NG = T // PG  # projection groups

    with tile.TileContext(nc) as tc, ExitStack() as ctx:
        constp = ctx.enter_context(tc.tile_pool(name="const", bufs=1))
        gpool = ctx.enter_context(tc.tile_pool(name="gam", bufs=2))
        hdp = ctx.enter_context(tc.tile_pool(name="hd", bufs=2))
        actp = ctx.enter_context(tc.tile_pool(name="act", bufs=2))
        hbgp = ctx.enter_context(tc.tile_pool(name="hbg", bufs=3))
        osbp = ctx.enter_context(tc.tile_pool(name="osb", bufs=2))
        pzrp = ctx.enter_context(tc.tile_pool(name="pzr", bufs=2, space="PSUM"))
        php = ctx.enter_context(tc.tile_pool(name="ph", bufs=2, space="PSUM"))
        pj01p = ctx.enter_context(tc.tile_pool(name="pj01", bufs=2, space="PSUM"))
        pj23p = ctx.enter_context(tc.tile_pool(name="pj23", bufs=2, space="PSUM"))

        wzr = constp.tile([128, KC * 2 * JT * 128], bf16)
        nc.sync.dma_start(wzr[:], wzr_d[:])
        wht = constp.tile([128, KC * JT * 128], bf16)
        nc.sync.dma_start(wht[:], wht_d[:])
        wlin = constp.tile([128, KC * JT * 128], bf16)
        nc.sync.dma_start(wlin[:], wlin_d[:])
        a0zr = constp.tile([128, 2 * FR], bf16)
        nc.sync.dma_start(a0zr[:], a0zr_d[:])
        a0h = constp.tile([128, FR], bf16)
        nc.sync.dma_start(a0h[:], a0h_d[:])
        blin01 = constp.tile([128, 2 * FR], bf16)
        nc.sync.dma_start(blin01[:], blin01_d[:])
        blin23 = constp.tile([128, 2 * FR], bf16)
        nc.sync.dma_start(blin23[:], blin23_d[:])
        ident = constp.tile([128, 128], bf16)
        nc.sync.dma_start(ident[:], ident_d[:])

        def wzr_blk(g, jo, kc):
            i = ((kc * 2 + g) * JT + jo) * 128
            return wzr[:, i:i + 128]

        def wht_blk(jo, kc):
            i = (kc * JT + jo) * 128
            return wht[:, i:i + 128]

        def wlin_blk(jo, kc):
            i = (kc * JT + jo) * 128
            return wlin[:, i:i + 128]

        # gamma chunks, preloaded half a chunk ahead
        chunks = {}

        def ensure_chunk(c):
            if c in chunks or c * GCH >= T:
                return
            t0 = c * GCH
            t1 = min(t0 + GCH, T)
            gt = gpool.tile([128, GCH * FR], bf16, tag="gchunk")
            nc.sync.dma_start(gt[:, 0:(t1 - t0) * FR], gam_d[:, t0:t1, :])
            chunks[c] = gt

        def gam_sl(tt, c0, c1):
            c2, o2 = divmod(tt, GCH)
            return chunks[c2][:, o2 * FR + c0:o2 * FR + c1]

        ensure_chunk(0)

        # step-0 decayed state is zero
        hdb = hdp.tile([128, FR], bf16, tag="hdb")
        nc.vector.memset(hdb[:], 0.0)

        mult = mybir.AluOpType.mult
        add = mybir.AluOpType.add

        hbfg = None       # current group's h stash (written steps 4g..4g+3)
        hbfg_done = None  # previous group's h stash (read by projection)
        pj01 = pj23 = None

        def proj_step(t):
            """Projection work scheduled during step t: round (t-4)%4 of
            group (t-4)//4, plus PSUM init / drain at the group edges."""
            nonlocal pj01, pj23
            if t < PG or t >= PG * (NG + 1):
                return
            g, rkc = divmod(t - PG, PG)
            if rkc == 0:
                pj01 = pj01p.tile([128, 2 * FR], f32, tag="pj01")
                pj23 = pj23p.tile([128, 2 * FR], f32, tag="pj23")
                nc.tensor.matmul(pj01[:], ident[:], blin01[:], start=True, stop=False)
                nc.tensor.matmul(pj23[:], ident[:], blin23[:], start=True, stop=False)
            src = hbfg_done.rearrange("p (s k b) -> p s k b", s=PG, k=KC, b=BL)
            for pair, pj in ((0, pj01), (1, pj23)):
                for joh in (0, 1):
                    jo = pair * 2 + joh
                    nc.tensor.matmul(
                        pj[:, joh * FR:(joh + 1) * FR],
                        wlin_blk(jo, rkc),
                        src[:, :, rkc],
                        start=False, stop=(rkc == KC - 1),
                    )

        def proj_drain(t):
            """After round 3 of group g (t = 4g+7): copy psum out and DMA."""
            if t < 2 * PG - 1 or (t - (2 * PG - 1)) % PG != 0:
                return
            g = (t - PG) // PG
            osb = osbp.tile([128, PG * FR], f32, tag="osb")
            dst = osb.rearrange("p (s q b) -> p q s b", s=PG, q=JT, b=BL)
            nc.scalar.copy(dst[:, 0:2], pj01[:])
            nc.vector.tensor_copy(dst[:, 2:4], pj23[:])
            nc.sync.dma_start(out_d[:, g * PG * FR:(g + 1) * PG * FR], osb[:])

        for t in range(T):
            c, o = divmod(t, GCH)
            if o == GCH // 2:
                ensure_chunk(c + 1)

            if t % PG == 0:
                hbfg, hbfg_done = hbgp.tile([128, PG * FR], bf16, tag="hbfg",
                                            name="hbfg"), hbfg

            # ---- per-step PSUM inits (fill the tail-drain stall of step t-1)
            pzr = pzrp.tile([128, 2 * FR], f32, tag="pzr")
            nc.tensor.matmul(pzr[:], ident[:], a0zr[:], start=True, stop=False)
            ph = php.tile([128, FR], f32, tag="ph")
            nc.tensor.matmul(ph[:], ident[:], a0h[:], start=True, stop=False)

            # ---- one contraction-round of the batched projection
            proj_step(t)

            # ---- r gate, kc-major: round kc consumes hdb chunk kc as the
            # tail of step t-1 produces it
            for kc in range(KC):
                for jo in range(JT):
                    nc.tensor.matmul(
                        pzr[:, FR + jo * BL:FR + (jo + 1) * BL],
                        wzr_blk(1, jo, kc),
                        hdb[:, kc * BL:(kc + 1) * BL],
                        start=False, stop=(kc == KC - 1),
                    )

            rb = actp.tile([128, FR], bf16, tag="rb")
            nc.scalar.activation(rb[:, 0:HB], pzr[:, FR:FR + HB], AF.Sigmoid)
            nc.scalar.activation(rb[:, HB:FR], pzr[:, FR + HB:2 * FR], AF.Sigmoid)
            rh = hdp.tile([128, FR], bf16, tag="rh")
            nc.vector.tensor_mul(rh[:, 0:HB], rb[:, 0:HB], hdb[:, 0:HB])
            nc.vector.tensor_mul(rh[:, HB:FR], rb[:, HB:FR], hdb[:, HB:FR])

            # ---- z gate kc 0,1 (fills the sigmoid->rh latency window)
            for kc in (0, 1):
                for jo in range(JT):
                    nc.tensor.matmul(
                        pzr[:, jo * BL:(jo + 1) * BL],
                        wzr_blk(0, jo, kc),
                        hdb[:, kc * BL:(kc + 1) * BL],
                        start=False, stop=False,
                    )
            # ---- candidate kc 0,1 (needs rh half 0 only)
            for kc in (0, 1):
                for jo in range(JT):
                    nc.tensor.matmul(
                        ph[:, jo * BL:(jo + 1) * BL],
                        wht_blk(jo, kc),
                        rh[:, kc * BL:(kc + 1) * BL],
                        start=False, stop=False,
                    )
            # ---- z gate kc 2,3
            for kc in (2, 3):
                for jo in range(JT):
                    nc.tensor.matmul(
                        pzr[:, jo * BL:(jo + 1) * BL],
                        wzr_blk(0, jo, kc),
                        hdb[:, kc * BL:(kc + 1) * BL],
                        start=False, stop=(kc == KC - 1),
                    )
            # ---- candidate kc 2,3
            for kc in (2, 3):
                for jo in range(JT):
                    nc.tensor.matmul(
                        ph[:, jo * BL:(jo + 1) * BL],
                        wht_blk(jo, kc),
                        rh[:, kc * BL:(kc + 1) * BL],
                        start=False, stop=(kc == KC - 1),
                    )

            zb = actp.tile([128, FR], bf16, tag="zb")
            nc.scalar.activation(zb[:], pzr[:, 0:FR], AF.Sigmoid)

            # ---- off-chain tail pieces (ready before tanh lands)
            zm = actp.tile([128, FR], bf16, tag="zm")
            nc.vector.tensor_scalar(zm[:], zb[:], -1.0, 1.0, mult, add)
            pp = actp.tile([128, FR], bf16, tag="pp")
            nc.vector.tensor_mul(pp[:], zm[:], hdb[:])
            last = t + 1 >= T
            if not last:
                cg = actp.tile([128, FR], bf16, tag="cg")
                nc.vector.tensor_mul(cg[:], gam_sl(t + 1, 0, FR), zb[:])
                ag = actp.tile([128, FR], bf16, tag="ag")
                nc.vector.tensor_mul(ag[:], gam_sl(t + 1, 0, FR), pp[:])
                hdb_n = hdp.tile([128, FR], bf16, tag="hdb")

            htl = actp.tile([128, FR], bf16, tag="htl")
            hsl = hbfg[:, (t % PG) * FR:(t % PG + 1) * FR]
            for hf in (0, 1):
                sl = slice(hf * HB, (hf + 1) * HB)
                nc.scalar.activation(htl[:, sl], ph[:, sl], AF.Tanh)
                if not last:
                    # on-chain: hdb' = c*htl + a  (two DVE ops per half)
                    ug = actp.tile([128, HB], bf16, tag=f"ug{hf}")
                    nc.vector.tensor_mul(ug[:], cg[:, sl], htl[:, sl])
                    nc.vector.tensor_add(hdb_n[:, sl], ug[:], ag[:, sl])
            # off-chain: undecayed h for the projection
            qq = actp.tile([128, FR], bf16, tag="qq")
            nc.vector.tensor_mul(qq[:], zb[:], htl[:])
            nc.vector.tensor_add(hsl, pp[:], qq[:])

            if not last:
                hdb = hdb_n

            proj_drain(t)

        # remaining projection groups after the recurrence
        hbfg_done = hbfg
        for t in range(T, PG * (NG + 1)):
            proj_step(t)
            proj_drain(t)

    nc.compile()
    _BUILD_CACHE["nc"] = nc
    return nc


def _host_prep(C, t, Wz, bz, Wr, br, Wh, bh, Wgh, bgh, Wlin, blin):
    """Build per-core input maps (all the precomputed, packed device tensors)."""
    bf = ml_dtypes.bfloat16

    s = Wgh.sum(axis=0)  # (H,)
    t3 = t[:, :, 0]  # (T,B)
    dt = np.concatenate([np.zeros((1, B), np.float32), t3[1:] - t3[:-1]], axis=0)
    # gamma (T,B,H)
    gam = np.exp(-np.maximum(dt[:, :, None] * s[None, None, :] + bgh[None, None, :], 0.0)).astype(np.float32)

    def gate_const(W, b):
        # C @ W_x + colsum(W_m) + b  -> (B,H)
        return C @ W[0:H] + (W[2 * H:3 * H].sum(axis=0) + b)[None, :]

    Az0 = gate_const(Wz, bz).astype(np.float32)
    Ar0 = gate_const(Wr, br).astype(np.float32)
    Ah0 = gate_const(Wh, bh).astype(np.float32)

    Wg = np.stack([Wz[H:2 * H], Wr[H:2 * H]])  # (2,H,H)
    # wzr packed: [k, (kc,g,jo,m)]
    wzr = Wg.reshape(2, KC, 128, JT, 128).transpose(2, 1, 0, 3, 4).reshape(128, KC * 2 * JT * 128)
    wht = Wh[H:2 * H].reshape(KC, 128, JT, 128).transpose(1, 0, 2, 3).reshape(128, KC * JT * 128)
    # wlin packed: [k, (kc,jo,oc)] -> stationary blocks [k-chunk, o-block]
    wlinp = Wlin.reshape(KC, 128, JT, 128).transpose(1, 0, 2, 3).reshape(128, KC * JT * 128)
    wzr = np.ascontiguousarray(wzr, dtype=bf)
    wht = np.ascontiguousarray(wht, dtype=bf)
    wlinp = np.ascontiguousarray(wlinp, dtype=bf)
    ident = np.eye(128, dtype=bf)

    # blin init tiles: [p, jo'*FR + s*BL + b] = blin[(pair*2+jo')*128 + p]
    blin4 = blin.reshape(JT, 128)  # [jo, p]
    def blin_pair(pair):
        v = blin4[2 * pair:2 * pair + 2]  # (2, 128)
        out = np.broadcast_to(v.transpose(1, 0)[:, :, None], (128, 2, FR))
        return np.ascontiguousarray(out.reshape(128, 2 * FR), dtype=bf)

    in_maps = []
    for i in range(NCORES):
        sl = slice(i * BL, (i + 1) * BL)
        gf = gam[:, sl, :]  # (T,BL,H)
        # gam packed: [p, t, kt*BL+b]
        gp = np.ascontiguousarray(
            gf.reshape(T, BL, KC, 128).transpose(3, 0, 2, 1).reshape(128, T, KC * BL), dtype=bf)

        def packA(A):
            return A[sl].reshape(BL, JT, 128).transpose(2, 1, 0).reshape(128, JT * BL)

        a0zr = np.ascontiguousarray(
            np.concatenate([packA(Az0), packA(Ar0)], axis=1), dtype=bf)

        in_maps.append({
            "gam": gp,
            "wzr": wzr,
            "wht": wht,
            "wlin": wlinp,
            "a0zr": a0zr,
            "a0h": np.ascontiguousarray(packA(Ah0), dtype=bf),
            "blin01": blin_pair(0),
            "blin23": blin_pair(1),
            "ident": ident,
        })
    return in_maps


def kernel(C, t, mask, Wz, bz, Wr, br, Wh, bh, Wgh, bgh, wgx, bgx, Wlin, blin,
           _trace=False, _trace_kwargs=None):
    C = np.asarray(C, np.float32)
    t = np.asarray(t, np.float32)
    nc = _build_program()
    in_maps = _host_prep(C, t,
                         np.asarray(Wz, np.float32), np.asarray(bz, np.float32),
                         np.asarray(Wr, np.float32), np.asarray(br, np.float32),
                         np.asarray(Wh, np.float32), np.asarray(bh, np.float32),
                         np.asarray(Wgh, np.float32), np.asarray(bgh, np.float32),
                         np.asarray(Wlin, np.float32), np.asarray(blin, np.float32))

    from concourse.bass_utils import run_bass_kernel_spmd
    res = run_bass_kernel_spmd(nc, in_maps, list(range(NCORES)),
                               trace=_trace, **(_trace_kwargs or {}))
    outs = []
    for i in range(NCORES):
        arr = res.results[i]["out"].reshape(128, T, JT, BL)
        # out[t, b, jo*128+p] = arr[p, t, jo, b]
        outs.append(arr.transpose(1, 3, 2, 0).reshape(T, BL, O))
    full = np.concatenate(outs, axis=1).astype(np.float32)  # (T,B,O)
    kernel._last_results = res
    return full
